# revision 109
# baseline (speedup 1.0000x reference)
"""Causal MHA on 8 trn2 NeuronCores — v3 (fp8 DoubleRow projections).

Sharding: core c -> batch b=c//4, head group g=c%4 (4 heads = 256 proj cols).
Host preps per-core transposed inputs; device computes the o_proj
partial product for its head group; host sums the 4 partials per batch.

v3 on top of v2's schedule: the four projection GEMMs (Q, K, V, o_proj)
run as fp8e4 DoubleRow matmuls — 2 k-tiles per instruction at 0.5
cycles/row = 4x the fp16 rate per k-tile.  Full fp16-grade accuracy is
kept with a 3-term hi/lo expansion (x~x_hi+x_lo, w~w_hi+w_lo, drop
lo*lo): 12 DR matmuls replace 8 fp16 ones per 8-k-tile chain = 0.75x
PE time.  Host-side prep (free) supplies x16=16*x and w64=64*w hi/lo
splits scaled into e4m3's normal range; all scale compensations fold
into existing constants (exp scale 2^-23, vp sums-column 2.0, final
o_proj copy x 1/512).  attn-out is split hi/lo on device (Pool copy +
DVE subtract) for the o_proj DR.  Scores and attn@V stay fp16 — every
single-fp8 variant of those measured over the 2e-2 tolerance.

Schedule notes: the SE (scores+exp) stream runs up to RUN tiles ahead of
attn@V through a 34-deep atile pool; projection fillers are emitted as
~0.64us half-chain units (a half-open PSUM chain pins the next ps_f pop)
so the scalar engine's exp cadence never sees a long matmul gap; chunk
x-DMAs are emitted lazily with the chunk's first projection unit to keep
emission order = dependency order; the last chunk's SE head-pairs are
interleaved so its final exps land before the tail.

All w tensors are host-packed partition-major so each weight DMA is one
contiguous run per partition (byte-rate floor instead of the 128-byte
descriptor rate), which pulls the head's delivery-bound stalls in.

Timeline-sim (the graded metric): 110759 ns vs 117221 ns v2 baseline;
device rel err 2.3e-3 (tolerance 2e-2).
"""

import os

import numpy as np
import ml_dtypes

import concourse.bass as bass
import concourse.mybir as mybir
import concourse.tile as tile
from concourse.bass_utils import run_bass_kernel_spmd

F32 = mybir.dt.float32
F16 = mybir.dt.float16
F8 = mybir.dt.float8e4
E4 = ml_dtypes.float8_e4m3
DR = mybir.MatmulPerfMode.DoubleRow

B, S, D, H, DK = 2, 2048, 1024, 16, 64
HC = 4          # heads per core
M = HC * DK     # 256 proj columns per core
NK = D // 128   # 8 contraction tiles for projections
NST = S // 128  # 16 sequence tiles
NSC = S // 512  # 4 sequence chunks
EXP_SCALE = 2.0 ** -23   # 0.125 / (16*64)^2  (q,k carry 1024x scale)
O_SCALE = 1.0 / 512.0    # attn-out 8x, o-weights 64x


def _emit(ctx, tc, io):
    nc = tc.nc
    Exp = mybir.ActivationFunctionType.Exp

    marks = _CACHE.setdefault("marks", [])

    def mark(label):
        # debug aid: consumes one instruction id to timestamp the emission
        # stream; analyzers map I-n -> enclosing label
        marks.append((int(nc.get_next_instruction_name()[2:]), label))

    wpool = ctx.enter_context(tc.tile_pool(name="wpool", bufs=1))
    big = ctx.enter_context(tc.tile_pool(name="big", bufs=1))
    at = ctx.enter_context(tc.tile_pool(name="at", bufs=34))
    sm = ctx.enter_context(tc.tile_pool(name="sm", bufs=4))
    osb = ctx.enter_context(tc.tile_pool(name="osb", bufs=4))
    obuf = ctx.enter_context(tc.tile_pool(name="obuf", bufs=8))
    ps_a = ctx.enter_context(tc.tile_pool(name="ps_a", bufs=2, space="PSUM"))
    ps_b = ctx.enter_context(tc.tile_pool(name="ps_b", bufs=2, space="PSUM"))
    ps_f = ctx.enter_context(tc.tile_pool(name="ps_f", bufs=2, space="PSUM"))

    # ---- packed fp8 input loads ----
    x_sb = {v: big.tile([128, NK, S], F8, name=f"x{v}", tag=f"x{v}")
            for v in ("h", "l")}
    x_dram = {v: io[f"x{v}"].rearrange("(k p) s -> p k s", p=128)
              for v in ("h", "l")}
    w_sb = {}
    for wname in ("wq", "wk", "wv"):
        for v in ("h", "l"):
            w_sb[wname + v] = wpool.tile(
                [128, NK, M], F8, name=wname + v, tag=wname + v
            )

    # load order tuned for first-SE latency; w tensors are host-packed
    # partition-major so each load is one contiguous run per partition
    # (128 descriptors at the byte-rate floor, not 1024 at the desc rate)
    def wload(wn):
        nc.sync.dma_start(
            out=w_sb[wn],
            in_=io[wn].rearrange("p (k m) -> p k m", k=NK),
        )

    tm_sb = wpool.tile([128, 128], F16, name="tm", tag="tm")

    def xload(v, c, engine=None):
        # Pool-issued DMAs take the SWDGE path: no HWDGE contention, so the
        # x chunks stream in parallel with the w loads on the SP queue
        eng = engine or nc.gpsimd
        eng.dma_start(
            out=x_sb[v][:, :, 512 * c : 512 * (c + 1)],
            in_=x_dram[v][:, :, 512 * c : 512 * (c + 1)],
        )

    def xload_half(v, kh):
        # chunk-0 k-quarters race in on both DMA queues so the first DR
        # terms (which need only k-pair 0) start before the chunk is full
        for q in range(2):
            eng = nc.sync if q == 0 else nc.gpsimd
            ks = slice(4 * kh + 2 * q, 4 * kh + 2 * q + 2)
            eng.dma_start(
                out=x_sb[v][:, ks, 0:512], in_=x_dram[v][:, ks, 0:512]
            )

    wload("wqh")
    xload_half("h", 0)
    xload_half("h", 1)
    wload("wql")
    wload("wkh")
    xload_half("l", 0)
    xload_half("l", 1)
    wload("wkl")
    nc.sync.dma_start(out=tm_sb, in_=io["trimask"])
    wload("wvh")
    wload("wvl")

    ow_sb = {}
    for v in ("h", "l"):
        ow_sb[v] = wpool.tile([128, 2, D], F8, name=f"ow{v}", tag=f"ow{v}")

    qt_sb = [big.tile([128, S], F16, name=f"qt{m}", tag=f"qt{m}") for m in range(2)]
    kt_sb = [big.tile([128, S], F16, name=f"kt{m}", tag=f"kt{m}") for m in range(2)]
    vp = [
        big.tile([128, HC, DK + 1], F16, name=f"vp{st}", tag=f"vp{st}")
        for st in range(NST)
    ]
    # outTp holds the transposed attn-out as packed (hi, lo) fp8 byte pairs
    # inside each f16 element; the o_proj DR reads strided fp8 views
    outTp = big.tile([128, 2, S], F16, name="outTp", tag="outTp")
    aoh = outTp.bitcast(F8)[:, :, 0::2]
    aol = outTp.bitcast(F8)[:, :, 1::2]

    def dr_chain(ps, lhs_pairs, rhs_pairs):
        """Emit a 3-term DoubleRow accumulation chain into `ps`.

        lhs_pairs/rhs_pairs: list of (hi_slice, lo_slice) per k-pair.
        Terms per k-pair: (lh,rh), (lh_lo? ...) -> (h,h), (l,h), (h,l).
        """
        n = len(lhs_pairs)
        idx = 0
        total = 3 * n
        for t in range(3):
            for kp in range(n):
                lh, ll = lhs_pairs[kp]
                rh, rl = rhs_pairs[kp]
                lhsT, rhs = ((lh, rh), (ll, rh), (lh, rl))[t]
                nc.tensor.matmul(
                    ps, lhsT=lhsT, rhs=rhs,
                    start=(idx == 0), stop=(idx == total - 1),
                    perf_mode=DR, skip_group_check=True,
                )
                idx += 1

    def qk_half(wname, dest, mt, c, half, holder):
        mark(f"qk:{wname}:mt{mt}:c{c}.{half}")
        if half == 0:
            holder["ps"] = ps_f.tile([128, 512], F32, name="psqk", tag="ps_f")
        ps = holder["ps"]
        sl = slice(512 * c + 256 * half, 512 * c + 256 * (half + 1))
        dr_chain(
            ps[:, 256 * half : 256 * (half + 1)],
            [(w_sb[wname + "h"][:, 2 * kp : 2 * kp + 2, 128 * mt : 128 * (mt + 1)],
              w_sb[wname + "l"][:, 2 * kp : 2 * kp + 2, 128 * mt : 128 * (mt + 1)])
             for kp in range(NK // 2)],
            [(x_sb["h"][:, 2 * kp : 2 * kp + 2, sl],
              x_sb["l"][:, 2 * kp : 2 * kp + 2, sl])
             for kp in range(NK // 2)],
        )
        if half == 1:
            nc.vector.tensor_copy(dest[mt][:, 512 * c : 512 * (c + 1)], ps)

    def qk_group(wname, dest, mt, c):
        holder = {}
        qk_half(wname, dest, mt, c, 0, holder)
        qk_half(wname, dest, mt, c, 1, holder)

    def v_group(st):
        mark(f"v:st{st}")
        ps = ps_f.tile([128, 512], F32, name="psv", tag="ps_f")
        sl = slice(128 * st, 128 * (st + 1))
        dr_chain(
            ps[:, 0:M],
            [(x_sb["h"][:, 2 * kp : 2 * kp + 2, sl],
              x_sb["l"][:, 2 * kp : 2 * kp + 2, sl])
             for kp in range(NK // 2)],
            [(w_sb["wvh"][:, 2 * kp : 2 * kp + 2, :],
              w_sb["wvl"][:, 2 * kp : 2 * kp + 2, :])
             for kp in range(NK // 2)],
        )
        # v16 = psum/64 ; sums column rides at 2.0 so normalize yields 8*ao
        nc.vector.tensor_scalar_mul(
            vp[st][:, :, 0:DK],
            ps[:, 0:M].rearrange("p (h d) -> p h d", h=HC),
            1.0 / 64.0,
        )
        nc.vector.memset(vp[st][:, :, DK : DK + 1], 2.0)

    def split_group(o_sb, op_sb, half):
        """Pack attn-out into (hi, lo) fp8 byte pairs, pre-transpose.

        Depends only on the normalize output, so it pipelines on DVE right
        behind it; the transposes then move the packed f16 elements."""
        mark(f"split:{half}")
        jp = slice(2 * half, 2 * half + 2)
        hi = op_sb.bitcast(F8)[:, jp, 0::2]
        nc.vector.tensor_copy(hi, o_sb[:, jp, :])
        nc.vector.tensor_tensor(
            out=op_sb.bitcast(F8)[:, jp, 1::2], in0=o_sb[:, jp, :], in1=hi,
            op=mybir.AluOpType.subtract,
        )

    obufs = {}

    def o_group(st, nck, psrc="f", copier="v"):
        mark(f"o:st{st}:n{nck}:{psrc}")
        if psrc == "a":  # tail only: borrow idle attention PSUM slots
            ps = ps_a.tile([128, 1024], F32, name="pso", tag="ps_a")[:, 0:512]
        elif psrc == "b":
            ps = ps_b.tile([128, 512], F32, name="pso", tag="psb")
        else:
            ps = ps_f.tile([128, 512], F32, name="pso", tag="ps_f")
        stl = slice(128 * st, 128 * (st + 1))
        for half in range(2):
            off = 512 * nck + 256 * half
            dr_chain(
                ps[:, 256 * half : 256 * (half + 1)],
                [(aoh[:, :, stl], aol[:, :, stl])],
                [(ow_sb["h"][:, :, off : off + 256],
                  ow_sb["l"][:, :, off : off + 256])],
            )
        if st not in obufs:
            obufs[st] = obuf.tile([128, 1024], F16, name="ob", tag="ob")
        ob = obufs.pop(st) if nck == 1 else obufs[st]
        obh = ob[:, 512 * nck : 512 * (nck + 1)]
        if copier == "s":  # tail only: scalar engine is idle after the last exp
            nc.scalar.mul(obh, ps, O_SCALE)
        else:
            nc.vector.tensor_scalar_mul(obh, ps, O_SCALE)
        if nck == 1:  # both halves written: one batched output DMA per st
            nc.sync.dma_start(
                out=io["out_p"][128 * st : 128 * (st + 1), :], in_=ob
            )

    # ---- attention as two decoupled streams ----
    tiles_seq = [
        (c, hp, u) for c in range(NSC) for hp in range(2) for u in range(4 * c + 4)
    ]
    # SE emission order: interleave the two head-pairs within the last chunk
    # so its hp1 exps (which gate the final AV tiles) finish earlier
    se_seq = [k for k in tiles_seq if k[0] < 3]
    for u in range(16):
        se_seq += [(3, 0, u), (3, 1, u)]
    atiles = {}
    osbs = {}
    psbs = {}

    def emit_se(key):
        c, hp, u = key
        mark(f"se:{c}.{hp}.{u}")
        j0 = max(0, u - 4 * c)
        sqlo = 128 * j0
        n = 512 - sqlo
        psa = ps_a.tile([128, 1024], F32, name="psa", tag="ps_a")
        for ho in range(2):
            p0 = 64 * ho
            nc.tensor.matmul(
                psa[:, 512 * ho : 512 * ho + n],
                lhsT=kt_sb[hp][p0 : p0 + 64, 128 * u : 128 * (u + 1)],
                rhs=qt_sb[hp][p0 : p0 + 64, 512 * c + sqlo : 512 * (c + 1)],
                start=True,
                stop=True,
            )
        atile = at.tile([128, 1024], F16, name="atile", tag="at")
        nc.scalar.activation(
            atile.rearrange("p (b x) -> p b x", b=2)[:, :, 0:n],
            psa.rearrange("p (b x) -> p b x", b=2)[:, :, 0:n],
            Exp,
            scale=EXP_SCALE,
        )
        if u >= 4 * c:  # diagonal tile: mask the 128-col block (on GPSIMD)
            for ho in range(2):
                nc.gpsimd.tensor_mul(
                    atile[:, 512 * ho : 512 * ho + 128],
                    atile[:, 512 * ho : 512 * ho + 128],
                    tm_sb,
                )
        atiles[key] = atile

    def emit_av(key):
        c, hp, u = key
        mark(f"av:{c}.{hp}.{u}")
        j0 = max(0, u - 4 * c)
        if u == 0:
            if hp == 0:
                osbs[c] = (
                    osb.tile([128, 4, M], F16, name="o_sb", tag="o_sb"),
                    osb.tile([128, 4, M], F16, name="op_sb", tag="op_sb"),
                )
            psbs[(c, hp)] = [
                ps_b.tile([128, 260], F32, name=f"psb{half}", tag="psb")
                for half in range(2)
            ]
        atile = atiles.pop(key)
        psb = psbs[(c, hp)]
        for j in range(j0, 4):
            bank = psb[j // 2]
            for ho in range(2):
                nc.tensor.matmul(
                    bank[:, 130 * (j % 2) + 65 * ho :][:, 0:65],
                    lhsT=atile[
                        :, 512 * ho + 128 * (j - j0) : 512 * ho + 128 * (j - j0) + 128
                    ],
                    rhs=vp[u][:, 2 * hp + ho, :],
                    start=(u == 0 and j % 2 == 0 and ho == 0),
                    stop=(u == 4 * c + j),
                    skip_group_check=True,
                )
        if u < 4 * c + 1 or u == 4 * c + 2:
            return
        # a psb bank's chains are complete as soon as its two sq-tiles' last
        # sk-tile lands: bank0 at u=4c+1, bank1 at u=4c+3 — normalize eagerly
        half = 0 if u == 4 * c + 1 else 1
        o_sb, op_sb = osbs[c]
        bank = psb[half]
        rec = sm.tile([128, 4], F32, name="rec", tag="rec")
        rec_src = bass.AP(
            tensor=bank.tensor, offset=bank.offset + 64,
            ap=[list(bank.ap[0]), [65, 4]],
        )
        nc.vector.reciprocal(rec, rec_src)
        out_ap = bass.AP(
            tensor=o_sb.tensor,
            offset=o_sb.offset + M * 2 * half + 128 * hp,
            ap=[list(o_sb.ap[0]), [M, 2], [64, 2], [1, 64]],
        )
        in0 = bass.AP(
            tensor=bank.tensor, offset=bank.offset,
            ap=[list(bank.ap[0]), [130, 2], [65, 2], [1, 64]],
        )
        in1 = bass.AP(
            tensor=rec.tensor, offset=rec.offset,
            ap=[list(rec.ap[0]), [2, 2], [1, 2], [0, 64]],
        )
        nc.vector.tensor_tensor(
            out=out_ap, in0=in0, in1=in1, op=mybir.AluOpType.mult
        )
        if hp == 1:  # both head-pairs done for these 2 sq-tiles
            split_group(o_sb, op_sb, half)
            teng = nc.sync
            for j in (2 * half, 2 * half + 1):
                # batched xbar transpose: both 128x128 head-tiles in one call
                teng.dma_start_transpose(
                    out=outTp[:, :, 128 * (4 * c + j) : 128 * (4 * c + j) + 128],
                    in_=op_sb[:, j, :],
                )

    # ---- flat interleaved schedule ----
    tune = _CACHE.get("tune", {})
    RUN = tune.get("RUN", 30)  # SE tiles the scalar engine may run ahead

    QK_COST, V_COST, O_COST = 1.28, 0.64, 0.32  # filler PE us

    def chunk_proj_groups(cc):
        # fine-grained filler units (each ~0.64us PE) so interleaved scores
        # keep the scalar engine's exp cadence without long gaps
        gs = []
        if cc >= 1:
            # the chunk's x DMAs must precede its first projection matmul in
            # EMISSION order (the tile framework orders by program position);
            # on the SP queue they land BEHIND the head-critical transfers
            # instead of preempting them on the serial DMA engines
            def xfirst(cc=cc):
                xload("h", cc, nc.sync)
                xload("l", cc, nc.sync)
            gs.append((0.0, xfirst, False))
        for mt in range(2):
            for wname, dest in (("wq", qt_sb), ("wk", kt_sb)):
                holder = {}
                for half in range(2):
                    # half 0 leaves an open PSUM chain: half 1 MUST be the
                    # next ps_f user or the pool recycles the open bank
                    gs.append((
                        QK_COST / 2,
                        lambda mt=mt, w=wname, d=dest, h=half, hd=holder:
                            qk_half(w, d, mt, cc, h, hd),
                        half == 0,
                    ))
        for st in range(4 * cc, 4 * cc + 4):
            gs.append((V_COST, lambda st=st: v_group(st), False))
        return gs

    # head: only the mt0 q/k groups go before the first SE tiles, so the
    # first scores/exps never queue behind v-chains waiting on late DMAs
    qk_group("wq", qt_sb, 0, 0)
    qk_group("wk", kt_sb, 0, 0)
    for u in range(4):
        emit_se((0, 0, u))
    qk_group("wq", qt_sb, 1, 0)
    qk_group("wk", kt_sb, 1, 0)
    for st in range(4):
        v_group(st)
    proj_emitted = [True, False, False, False]
    proj_q = []
    for cc in range(1, NSC):
        proj_q.extend((cc, cg, gg, op) for cg, gg, op in chunk_proj_groups(cc))
    o_q = []

    def emit_proj_until(cse):
        cost = 0.0
        while not proj_emitted[cse] and proj_q:
            cc, cg, g, op = proj_q.pop(0)
            g()
            cost += cg * 1000.0
            if not proj_q or proj_q[0][0] != cc:
                proj_emitted[cc] = True
            if proj_emitted[cse] and not op:
                break
        return cost

    # filler PE-us to release per AV step, by chunk
    CREDIT = tune.get("CREDIT", [0.60, 0.45, 0.40, 0.25])

    O_DELAY = tune.get("O_DELAY", 12)  # AV steps before an st-pair's o_groups
    pending_o = []  # (release_at_av_i, [(st, nck), ...])

    se_i = 4
    av_i = 0
    filler_credit = 0.0
    chain_open = False
    while av_i < len(tiles_seq):
        while se_i < len(se_seq) and se_i - av_i < RUN:
            cse = se_seq[se_i][0]
            if not proj_emitted[cse]:
                emit_proj_until(cse)  # force-emit the projections SE needs
            emit_se(se_seq[se_i])
            se_i += 1
        key = tiles_seq[av_i]
        emit_av(key)
        av_i += 1
        c, hp, u = key
        if av_i == 4:  # AV inside chunk 0: stream in the o_proj weights
            for v in ("h", "l"):
                nc.sync.dma_start(
                    out=ow_sb[v],
                    in_=io[f"ow{v}"].rearrange("p (k n) -> p k n", k=2),
                )
        if hp == 1 and (u == 4 * c + 1 or u == 4 * c + 3):
            base = 4 * c if u == 4 * c + 1 else 4 * c + 2
            # c3's AV steps are ACT-paced (~2x longer), so the transpose
            # chain needs fewer steps of cover there — and the earlier
            # release fills the exp-bound window with o_proj work
            od = O_DELAY if c < 3 else tune.get("O_DELAY3", 6)
            pending_o.append(
                (av_i + od, [(st, nck) for st in (base, base + 1)
                             for nck in range(2)])
            )
        while pending_o and pending_o[0][0] <= av_i:
            o_q.extend(pending_o.pop(0)[1])
        filler_credit += CREDIT[c]
        while (filler_credit > 0 or chain_open) and (proj_q or o_q):
            if proj_q:
                cc, cg, g, op = proj_q.pop(0)
                g()
                chain_open = op
                if not proj_q or proj_q[0][0] != cc:
                    proj_emitted[cc] = True
                filler_credit -= cg
                if chain_open:
                    break  # half1 is forced first thing next AV step
            else:
                st, nck = o_q.pop(0)
                o_group(st, nck)
                filler_credit -= O_COST
    for _, _, g, _ in proj_q:
        g()
    while pending_o:
        o_q.extend(pending_o.pop(0)[1])
    # tail flush: the scalar engine is idle (exps done) and the attention
    # PSUM pools are free — spread the last o_proj groups across all three
    # pools and copy their outputs on ACT to multiply the in-flight chains
    rot = ["f", "a", "b"]
    i = 0
    while o_q:
        st, nck = o_q.pop(0)
        o_group(st, nck, psrc=rot[i % 3], copier="s" if i % 2 else "v")
        i += 1


def _legalize_single_wait(nc):
    """The cayman TPB instruction struct has one embedded wait slot, and this
    walrus build refuses instructions with more. Hoist extra waits onto
    injected same-engine NoOps directly before each instruction — engine
    queues are strict FIFO, so semantics are preserved."""
    f = nc.m.functions[0]
    for blk in f.blocks:
        insts = blk.instructions  # live list
        i = 0
        while i < len(insts):
            ins = insts[i]
            si = ins.sync_info
            if si is not None and si.on_wait and len(si.on_wait) > 1:
                waits = list(si.on_wait)
                for w in waits[:-1]:
                    nop = mybir.InstNoOp(
                        name=nc.get_next_instruction_name(),
                        engine=ins.engine,
                        bass_nofuse=True,
                        sync_info=mybir.SyncInfo(on_wait=[w], on_update=[]),
                    )
                    nc.register_instruction(nop)
                    insts.insert(i, nop)
                    i += 1
                ins.sync_info = mybir.SyncInfo(
                    on_wait=[waits[-1]], on_update=list(si.on_update or [])
                )
            i += 1


_CACHE = {}


def _build():
    if "nc" in _CACHE:
        return _CACHE["nc"]
    nc = bass.Bass(
        "TRN2",
        target_bir_lowering=False,
        debug=False,
        enable_asserts=False,
        num_devices=8,
    )
    io = {
        "xh": nc.dram_tensor("xh", (D, S), F8, kind="ExternalInput").ap(),
        "xl": nc.dram_tensor("xl", (D, S), F8, kind="ExternalInput").ap(),
        "wqh": nc.dram_tensor("wqh", (128, NK * M), F8, kind="ExternalInput").ap(),
        "wql": nc.dram_tensor("wql", (128, NK * M), F8, kind="ExternalInput").ap(),
        "wkh": nc.dram_tensor("wkh", (128, NK * M), F8, kind="ExternalInput").ap(),
        "wkl": nc.dram_tensor("wkl", (128, NK * M), F8, kind="ExternalInput").ap(),
        "wvh": nc.dram_tensor("wvh", (128, NK * M), F8, kind="ExternalInput").ap(),
        "wvl": nc.dram_tensor("wvl", (128, NK * M), F8, kind="ExternalInput").ap(),
        "owh": nc.dram_tensor("owh", (128, 2 * D), F8, kind="ExternalInput").ap(),
        "owl": nc.dram_tensor("owl", (128, 2 * D), F8, kind="ExternalInput").ap(),
        "trimask": nc.dram_tensor(
            "trimask", (128, 128), F16, kind="ExternalInput"
        ).ap(),
        "out_p": nc.dram_tensor("out_p", (S, D), F16, kind="ExternalOutput").ap(),
    }
    from contextlib import ExitStack

    with tile.TileContext(nc) as tc, ExitStack() as ctx:
        _emit(ctx, tc, io)
    _legalize_single_wait(nc)
    _CACHE["nc"] = nc
    return nc


def _split8(t):
    hi = t.astype(E4)
    lo = (t - hi.astype(np.float32)).astype(E4)
    return hi, lo


def make_in_maps(x, qw, kw, vw, ow):
    x = np.asarray(x, dtype=np.float32)
    qw = np.asarray(qw, dtype=np.float32)
    kw = np.asarray(kw, dtype=np.float32)
    vw = np.asarray(vw, dtype=np.float32)
    ow = np.asarray(ow, dtype=np.float32)
    trimask = np.triu(np.ones((128, 128))).astype(np.float16)
    xsp = [_split8(np.ascontiguousarray(16.0 * x[b].T)) for b in range(2)]

    def _pack(t):
        # [K, N] -> partition-major [128, (K//128)*N]: one contiguous DMA
        # run per partition instead of (K//128)*128 short ones
        k, n = t.shape
        return np.ascontiguousarray(
            t.reshape(k // 128, 128, n).transpose(1, 0, 2).reshape(128, -1)
        )

    in_maps = []
    for c in range(8):
        b, g = c // 4, c % 4
        sl = slice(M * g, M * (g + 1))
        wqh, wql = map(_pack, _split8(np.ascontiguousarray(64.0 * qw[sl].T)))
        wkh, wkl = map(_pack, _split8(np.ascontiguousarray(64.0 * kw[sl].T)))
        wvh, wvl = map(_pack, _split8(np.ascontiguousarray(64.0 * vw[sl].T)))
        owh, owl = map(_pack, _split8(np.ascontiguousarray(64.0 * ow[:, sl].T)))
        in_maps.append(
            {
                "xh": xsp[b][0], "xl": xsp[b][1],
                "wqh": wqh, "wql": wql,
                "wkh": wkh, "wkl": wkl,
                "wvh": wvh, "wvl": wvl,
                "owh": owh, "owl": owl,
                "trimask": trimask,
            }
        )
    return in_maps


def kernel(x, q_proj_weight, k_proj_weight, v_proj_weight, o_proj_weight):
    nc = _build()
    in_maps = make_in_maps(
        x, q_proj_weight, k_proj_weight, v_proj_weight, o_proj_weight
    )
    trace = bool(os.environ.get("KERNEL_TRACE"))
    if trace:
        try:
            from antenv.axon_hooks import get_axon_ntff_profile_hook  # noqa: F401
        except ImportError:
            trace = False
    res = run_bass_kernel_spmd(
        nc, in_maps, core_ids=list(range(8)), trace=trace
    )
    if trace and res.exec_time_ns is not None:
        print(f"HW exec time: {res.exec_time_ns} ns")
        print(f"mean exec time: {res.mean_exec_time_ns} ns")
    parts = [r["out_p"].astype(np.float32) for r in res.results]
    out = np.stack(
        [
            parts[0] + parts[1] + parts[2] + parts[3],
            parts[4] + parts[5] + parts[6] + parts[7],
        ],
        axis=0,
    )
    return out
